# revision 75
# baseline (speedup 1.0000x reference)
"""Multi-head attention (B=2, S=2048, D=1024, H=16) on 8 Trainium2 NeuronCores.

Sharding: core c handles batch b = c//4 and head group g = c%4 (4 heads = 2
head-pairs, 256 model dims).  Each core computes q/k/v projections for its
heads, attention, and a row-parallel partial output projection; the host sums
the 4 partials per batch and adds the bias.

All tensors are fp16 (1 cycle/row on the PE at any moving width, vs f32r's
>=256 requirement), halving DMA and SBUF vs f32.  Layouts:
  xT   [d, s]        (host pre-transposed)
  qT/kT [e, s]       per head-pair tile (head 2m at partitions 0-63, 2m+1 at
                     64-127) so the scores contraction dim (hd=64) sits on
                     partitions
  v    [ks, ksb, h, hd+1]  with a trailing ones column: the PV matmul's 65th
                     output column is the softmax denominator
  scores [ks, qs]    per head; exp'd tiles pt feed PV as the STATIONARY
                     operand (moving = v, N=65) so PV charges N=65/row
                     instead of N=qs -- half the cost of the [e, qs]
                     orientation
  ctx  [qs, e]       normalized via a per-partition reciprocal multiply (the
                     denominator lands on the qs partition axis -- no DRAM
                     broadcast round-trip), then block-transposed to [e, qs]
                     by a DMA xbar transpose for the output projection.

Softmax needs no max-subtraction (scores ~ N(0,1)).  exp splits across
engines per tile: ACT runs native Exp on cols [0:416); DVE computes cols
[416:512) with a Schraudolph bit-trick (y = int16(x*1024/ln2 + 15330)
bitcast to fp16, ~1.8% RMS sawtooth on 18.75% of weights) so the softmax
never throttles the PE.

PSUM (8 banks): shared [128,512] ring "gp" (2) for qk/v/proj groups, scores
pair ring [128,2,512]x2 (4), ctx accumulators [128,2,130]x2 (2).

Cost-model makespan per core target: ~150 us (PE-bound: QKV 41us + scores
54.6us + PV 27.7us + proj 13.7us at 2.4GHz warm).
"""

import os
import sys

import numpy as np

for _p in ("/opt/trn_rl_repo", "/root/.axon_site/_ro/trn_rl_repo"):
    if os.path.isdir(_p) and _p not in sys.path:
        sys.path.insert(0, _p)

import bass_rust
import concourse.bass as bass
import concourse.mybir as mybir
import concourse.tile as tile
from concourse.bass_utils import run_bass_kernel_spmd
from concourse.vector_clock import ScopedClock, VectorClock
from contextlib import ExitStack

F32 = mybir.dt.float32
F16 = mybir.dt.float16
I16 = mybir.dt.int16
EXP = mybir.ActivationFunctionType.Exp
MULT = mybir.AluOpType.mult
ADD = mybir.AluOpType.add

B = 2
S = 2048
D = 1024
H = 16
HD = 64
NCORES = 8
GROUPS = 4          # head groups (cores per batch)
HG = H // GROUPS    # heads per core = 4
E = HG * HD         # head dims per core = 256
KT = D // 128       # contraction tiles over model dim = 8
SB = S // 128       # 128-row s blocks = 16
NQ = 4              # qs quarters (512 wide)

# exp engine split within each [128, 2, 512] scores tile: the scores MMs
# write qs-128-blocks rotated by ksb, so ACT always runs native Exp on the
# contiguous sc cols [0:384) (one instruction) and DVE runs a Schraudolph
# bit-trick on [384:512) -- every qs block gets the approximation on exactly
# 4/16 of its ks blocks.
SCH_A = float(1024.0 / np.log(2.0))
SCH_B = 15360.0 - 30.0

_carrier_counter = [0]


def _split_multi_waits(ordered):
    """This walrus build allows one sync wait per instruction; Tile's wait
    assignment can attach several.  Hoist extras onto same-engine InstNoOp
    carriers placed immediately before the instruction."""
    for bb_name, insts in ordered.items():
        new_list = []
        for inst in insts:
            si = inst.sync_info
            waits = list(si.on_wait) if si is not None else []
            if len(waits) > 1:
                for w in waits[:-1]:
                    _carrier_counter[0] += 1
                    carrier = mybir.InstNoOp(
                        name=f"I-waitc-{_carrier_counter[0]}", ins=[], outs=[]
                    )
                    carrier.engine = inst.engine
                    carrier.sync_info = bass_rust.SyncInfo(on_wait=[w], on_update=[])
                    new_list.append(carrier)
                inst.sync_info = bass_rust.SyncInfo(
                    on_wait=[waits[-1]],
                    on_update=list(si.on_update) if si is not None else [],
                )
            new_list.append(inst)
        ordered[bb_name] = new_list


class _TileContext(tile.TileContext):
    """TileContext adapted to the one-sync-wait-per-instruction walrus."""

    def _lower_ordered_insts(self, ordered):
        _split_multi_waits(ordered)
        return super()._lower_ordered_insts(ordered)

    def _drain_and_barrier(self, tick_clock, wait_clock):
        gc = tick_clock.global_clock
        for proc in range(len(gc)):
            if gc[proc] <= 0:
                continue
            cur = VectorClock([0 if i == proc else gc[i] for i in range(len(gc))])
            nop = self.nc.sync.nop()
            wait_clock.add_sem_waits(
                nop.ins, ScopedClock({None: gc}), ScopedClock({None: cur})
            )
        drain_inst = self.nc.sync.drain()
        wait_clock.add_sem_waits(
            drain_inst.ins, ScopedClock({None: gc}), ScopedClock({None: gc.copy()})
        )
        self.nc.all_engine_barrier()
        assert self.sems is not None
        popped = self.nc._tile_sem_poison_stack.pop()
        assert popped is self._sem_poison
        self.nc.clear_and_free_semaphores(list(self.sems.allocated().values()))
        self.nc.all_engine_barrier()


def build_nc(reps=1):
    nc = bass.Bass()
    xT = nc.declare_dram_parameter("xT", [D, S], F16, isOutput=False)
    # wqkv cols: k pair0 | k pair1 | q pair0 | q pair1 | v (4 heads x 64)
    wqkv = nc.declare_dram_parameter("wqkv", [D, 768], F16, isOutput=False)
    woT = nc.declare_dram_parameter("woT", [E, D], F16, isOutput=False)
    out = nc.declare_dram_parameter("out_partial", [S, D], F16, isOutput=True)

    with _TileContext(nc) as tc, ExitStack() as outer:
      for _rep in range(reps):
        ctx = outer.enter_context(ExitStack())
        act_pool = ctx.enter_context(tc.tile_pool(name="acts", bufs=1))
        x_sb = act_pool.tile([128, KT, S], F16, tag="x")
        w_sb = act_pool.tile([128, KT, 768], F16, tag="w")
        wo_sb = act_pool.tile([128, 2, D], F16, tag="wo")
        qT = [act_pool.tile([128, S], F16, tag=f"qT{m}", name=f"qT{m}") for m in range(2)]
        # kT is stored zero-padded per head (version r has head r's rows,
        # zeros elsewhere) so the scores matmuls contract K=128 at tile
        # (0,0): mixing fp16 matmuls across PE row-tile offsets 0/64 hangs
        # the hardware.  The paired qT stays as the moving operand -- the
        # stationary zeros cancel the other head's rows.
        kT = [act_pool.tile([128, 2, S], F16, tag=f"kT{m}", name=f"kT{m}") for m in range(2)]
        v_sb = act_pool.tile([128, SB, HG, HD + 1], F16, tag="v")
        ctx_sb = [act_pool.tile([128, SB, 128], F16, tag=f"cs{m}", name=f"cs{m}") for m in range(2)]
        ctxT = [act_pool.tile([128, SB, 128], F16, tag=f"ct{m}", name=f"ct{m}") for m in range(2)]

        # ---- input DMAs, ordered for earliest PE start: kT0 weights, x
        # chunk 0, then the rest ----
        def dma_w(c0, c1):
            nc.sync.dma_start(
                w_sb[:, :, c0:c1],
                wqkv[:, c0:c1].rearrange("(k p) e -> p k e", p=128),
            )

        def dma_x(c, s0=0, s1=512):
            nc.sync.dma_start(
                x_sb[:, :, c * 512 + s0:c * 512 + s1],
                xT[:, c * 512 + s0:c * 512 + s1].rearrange(
                    "(k p) s -> p k s", p=128
                ),
            )

        # k-pair-0 weights split by kt halves so the first accumulation
        # group's kt0-3 matmuls start ~1us earlier
        nc.sync.dma_start(
            w_sb[:, 0:4, 0:128],
            wqkv[0:512, 0:128].rearrange("(k p) e -> p k e", p=128),
        )
        nc.sync.dma_start(
            w_sb[:, 4:8, 0:128],
            wqkv[512:1024, 0:128].rearrange("(k p) e -> p k e", p=128),
        )
        dma_x(0, 0, 256)
        dma_x(0, 256, 512)
        dma_w(256, 384)      # q pair 0
        dma_x(1)
        dma_w(512, 768)      # v
        dma_w(128, 256)      # k pair 1
        dma_x(2)
        dma_w(384, 512)      # q pair 1
        dma_x(3)
        nc.sync.dma_start(
            wo_sb[:, :, :], woT[:, :].rearrange("(m p) e -> p m e", p=128)
        )
        # ones column for the softmax-denominator rows of v
        nc.gpsimd.memset(v_sb[:, :, :, HD], 1.0)
        # zero the padded halves of the kT versions (Pool; SBUF only)
        for m in range(2):
            for r in range(2):
                nc.gpsimd.memset(kT[m][64 * (1 - r):64 * (1 - r) + 64, r, :], 0.0)

        # ---- shared PSUM rings (8 banks: sc 2x2, sc_d 1, cx 2, gp 1) ----
        ps_gp = ctx.enter_context(tc.tile_pool(name="ps_gp", bufs=1, space="PSUM"))
        ps_sc = ctx.enter_context(tc.tile_pool(name="ps_sc", bufs=2, space="PSUM"))
        ps_scd = ctx.enter_context(tc.tile_pool(name="ps_scd", bufs=1, space="PSUM"))
        ps_cx = ctx.enter_context(tc.tile_pool(name="ps_cx", bufs=1, space="PSUM"))
        pt_pool = ctx.enter_context(tc.tile_pool(name="pt", bufs=8))
        ptd_pool = ctx.enter_context(tc.tile_pool(name="ptd", bufs=8))
        rc_pool = ctx.enter_context(tc.tile_pool(name="rc", bufs=4))
        st_pool = ctx.enter_context(tc.tile_pool(name="st", bufs=6))

        # GPSIMD cannot touch PSUM, so every PSUM exit goes through ACT or
        # DVE: ACT takes the early copies (its exp chain hasn't started),
        # DVE the rest (interleaved between Schraudolphs)
        def copy(dst, src, eng=None):
            eng = eng or nc.vector
            if eng is nc.scalar:
                eng.copy(dst, src)
            else:
                eng.tensor_copy(dst, src)

        pre_i = [0]

        def pre_ps():
            # pre-attention groups rotate through the idle sc/scd-ring banks
            # as well as gp, a 4-deep ring that hides the copy WAR chain
            pre_i[0] += 1
            ph = pre_i[0] % 4
            if ph == 0:
                return ps_gp.tile([128, 512], F32, tag="gp", name="qk")
            if ph == 3:
                return ps_scd.tile([128, 512], F32, tag="scd", name="qk")
            t = ps_sc.tile([128, 2, 384], F32, tag="sc", name="qk")
            return t.rearrange("p a b -> p (a b)")[:, 0:512]

        def qk_group(w_off, dst, c, width=512, off=0, eng=None, pad_k=False,
                     pre=False):
            sl = slice(c * 512 + off, c * 512 + off + width)
            ps = pre_ps() if pre else ps_gp.tile(
                [128, 512], F32, tag="gp", name="qk")
            for k in range(KT):
                nc.tensor.matmul(
                    ps[:, 0:width],
                    w_sb[:, k, w_off:w_off + 128],
                    x_sb[:, k, sl],
                    start=(k == 0),
                    stop=(k == KT - 1),
                )
            if pad_k:
                # split the pair rows into the two zero-padded kT versions
                copy(dst[0:64, 0, sl], ps[0:64, 0:width], eng)
                copy(dst[64:128, 1, sl], ps[64:128, 0:width], eng)
            else:
                copy(dst[:, sl], ps[:, 0:width], eng)

        def v_group(sb):
            ps = pre_ps()
            for k in range(KT):
                nc.tensor.matmul(
                    ps[:, 0:256],
                    x_sb[:, k, sb * 128:(sb + 1) * 128],
                    w_sb[:, k, 512:768],
                    start=(k == 0),
                    stop=(k == KT - 1),
                )
            copy(
                v_sb[:, sb, :, 0:HD],
                ps[:, 0:256].rearrange("p (h e) -> p h e", h=HG),
                nc.scalar,
            )

        def attention_phase(m, Q, carried_norms=()):
            # scores qs-block q lands at column block w = (q + ksb) % 4.
            # Blocks 0-2 go to the sc tile (read ONLY by ACT's native exp);
            # block 3 goes to the separate 1-bank sc_d tile (read ONLY by
            # DVE's Schraudolph): Tile serializes same-PSUM-tile readers, so
            # the reader split keeps ACT's exp chain free of the DVE.  The
            # rotation gives every qs column the approximation on exactly
            # 4/16 of its ks blocks.
            cx = [
                ps_cx.tile([128, 2, 2 * (HD + 1)], F32, tag=f"cx{j}", name=f"cx{j}")
                for j in range(2)
            ]
            # software-pipelined by 2: the static in-order PE queue must see
            # scores(t) BEFORE PV(t-2); otherwise a PV stalled on its exp
            # blocks the independent scores behind it and the PE idles for
            # most of every ACT instruction
            pts = {}
            for t in range(SB + 2):
                # the previous phase's normalize ops slot into DVE's per-
                # iteration slack here, never blocking a Schraudolph long
                if t - 1 < len(carried_norms) and t >= 1:
                    carried_norms[t - 1]()
                if t < SB:
                    ksb = t
                    sc = ps_sc.tile([128, 2, 384], F32, tag="sc", name="sc")
                    scd = ps_scd.tile([128, 2, 128], F32, tag="scd", name="scd")
                    for w in (0, 1, 2, 3):
                        q = (w - ksb) % 4
                        for r in range(2):
                            out_ap = (
                                scd[:, r, :] if w == 3
                                else sc[:, r, w * 128:(w + 1) * 128]
                            )
                            nc.tensor.matmul(
                                out_ap,
                                kT[m][:, r, ksb * 128:(ksb + 1) * 128],
                                qT[m][:, Q * 512 + q * 128:Q * 512 + (q + 1) * 128],
                                start=True,
                                stop=True,
                            )
                    pt = pt_pool.tile([128, 2, 384], F16, tag="pt", name="pt")
                    ptd = ptd_pool.tile([128, 2, 128], F16, tag="ptd", name="ptd")
                    nc.scalar.activation(pt[:, :, :], sc[:, :, :], EXP)
                    if os.environ.get("KDBG_NO_SCH"):
                        nc.vector.tensor_copy(ptd[:, :, :], scd[:, :, :])
                    else:
                        nc.vector.tensor_scalar(
                            ptd[:, :, :].bitcast(I16),
                            scd[:, :, :],
                            SCH_A,
                            SCH_B,
                            MULT,
                            ADD,
                        )
                    pts[ksb] = (pt, ptd)
                if t >= 2 and not os.environ.get("KDBG_NO_PV"):
                    ksb = t - 2
                    pt, ptd = pts.pop(ksb)
                    # one accumulation group per cx tile (2KB zero region):
                    # start only on the tile's first MM, stop on its last
                    for j in range(2):
                        for qq in range(2):
                            for r in range(2):
                                w = (j * 2 + qq + ksb) % 4
                                stat = (
                                    ptd[:, r, :] if w == 3
                                    else pt[:, r, w * 128:(w + 1) * 128]
                                )
                                nc.tensor.matmul(
                                    cx[j][:, qq, r * (HD + 1):(r + 1) * (HD + 1)],
                                    stat,
                                    v_sb[:, ksb, 2 * m + r, :],
                                    start=(ksb == 0 and qq == 0 and r == 0),
                                    stop=(ksb == SB - 1 and qq == 1 and r == 1),
                                )
            # normalize: ctx = ctxU * (1/den); den is PV output col 64 per
            # head (a per-partition scalar on the qs axis), then xbar-
            # transpose each 2-qsb block to [e, qs] for the projection.
            # Returned as closures the NEXT phase emits into its DVE slack.
            def norm_part(j, qq, with_recip, with_transpose, rc_box={}):
                def emit():
                    if os.environ.get("KDBG_NO_NORM"):
                        return
                    if with_recip:
                        rc = rc_pool.tile([128, 2, 2], F32, tag="rc", name="rc")
                        nc.vector.reciprocal(
                            rc[:], cx[j][:, :, HD:2 * (HD + 1):HD + 1]
                        )
                        rc_box[j] = rc
                    rc = rc_box[j]
                    qsb = Q * 4 + j * 2 + qq
                    nc.vector.tensor_mul(
                        ctx_sb[m][:, qsb, :].rearrange("p (h e) -> p h e", h=2),
                        cx[j][:, qq, :].rearrange(
                            "p (h e) -> p h e", e=HD + 1
                        )[:, :, 0:HD],
                        rc[:, qq, :].broadcast_to([128, 2, HD]),
                    )
                    if with_transpose:
                        if os.environ.get("KDBG_NO_TRANSPOSE"):
                            nc.sync.dma_start(
                                ctxT[m][:, Q * 4 + 2 * j:Q * 4 + 2 * j + 2, :],
                                ctx_sb[m][:, Q * 4 + 2 * j:Q * 4 + 2 * j + 2, :],
                            )
                        else:
                            nc.sync.dma_start_transpose(
                                ctxT[m][:, Q * 4 + 2 * j:Q * 4 + 2 * j + 2, :],
                                ctx_sb[m][:, Q * 4 + 2 * j:Q * 4 + 2 * j + 2, :],
                            )
                return emit

            box = {}
            return [
                norm_part(0, 0, True, False, box),
                norm_part(0, 1, False, True, box),
                norm_part(1, 0, True, False, box),
                norm_part(1, 1, False, True, box),
            ]

        def proj_half(Q, j, tail=False):
            for qsb in range(Q * 4 + 2 * j, Q * 4 + 2 * j + 2):
                stage = st_pool.tile([128, D], F16, tag="st", name="st")
                for nb in range(2):
                    # the final quarter's projections use the freed cx banks
                    # so the tail is double-buffered despite gp bufs=1
                    pso = (
                        ps_cx.tile([128, 512], F32, tag=f"cx{nb}", name="o")
                        if tail
                        else ps_gp.tile([128, 512], F32, tag="gp", name="o")
                    )
                    for m in range(2):
                        nc.tensor.matmul(
                            pso[:],
                            ctxT[m][:, qsb, :],
                            wo_sb[:, m, nb * 512:(nb + 1) * 512],
                            start=(m == 0),
                            stop=(m == 1),
                        )
                    copy(stage[:, nb * 512:(nb + 1) * 512], pso[:],
                         nc.scalar if tail else nc.vector)
                    if tail:
                        nc.sync.dma_start(
                            out[qsb * 128:(qsb + 1) * 128,
                                nb * 512:(nb + 1) * 512],
                            stage[:, nb * 512:(nb + 1) * 512],
                        )
                if not tail:
                    nc.sync.dma_start(out[qsb * 128:(qsb + 1) * 128, :], stage[:])

        # ---- emission order = scheduler priority: the minimum needed for
        # attention (m0, Q0) first, then the deferred q projections and
        # output projections as PE gap filler while exp chains bound the
        # attention phases ----
        # chunk-major so no x-gated group ever sits ahead of ready work;
        # the pre-attention copies run on ACT (its exp chain is idle there)
        qk_group(0, kT[0], 0, 256, eng=nc.scalar, pad_k=True, pre=True)
        qk_group(0, kT[0], 0, 256, 256, eng=nc.scalar, pad_k=True, pre=True)
        qk_group(256, qT[0], 0, 256, eng=nc.scalar, pre=True)
        qk_group(256, qT[0], 0, 256, 256, eng=nc.scalar, pre=True)
        qk_group(128, kT[1], 0, eng=nc.scalar, pad_k=True, pre=True)
        qk_group(384, qT[1], 0, eng=nc.scalar, pre=True)
        for sb in range(4):
            v_group(sb)
        for c in range(1, NQ):
            qk_group(0, kT[0], c, eng=nc.scalar, pad_k=True, pre=True)
            for sb in range(c * 4, c * 4 + 4):
                v_group(sb)
        NPH = int(os.environ.get("KDBG_NPHASES", "8"))
        phase_list = [(0, 0), (1, 0), (0, 1), (1, 1), (0, 2), (1, 2),
                      (0, 3), (1, 3)][:NPH]
        pi = [0]

        def next_phase(norms):
            if pi[0] >= len(phase_list):
                return norms
            m, Q = phase_list[pi[0]]
            pi[0] += 1
            return attention_phase(m, Q, norms)

        norms = next_phase(())                     # (0,0)
        for c in range(1, NQ):
            qk_group(128, kT[1], c, pad_k=True)
        qk_group(256, qT[0], 1)
        norms = next_phase(norms)                  # (1,0)
        qk_group(384, qT[1], 1)
        norms = next_phase(norms)                  # (0,1)
        if NPH >= 3:
            proj_half(0, 0)
        qk_group(256, qT[0], 2)
        if NPH >= 3:
            proj_half(0, 1)
        for Q in range(1, NQ):
            norms = next_phase(norms)              # (1,Q)
            if Q < NQ - 1:
                qk_group(384, qT[1], Q + 1)
                norms = next_phase(norms)          # (0,Q+1)
                if NPH >= 2 * Q + 3:
                    proj_half(Q, 0)
                if Q < NQ - 2:
                    qk_group(256, qT[0], Q + 2)
                if NPH >= 2 * Q + 3:
                    proj_half(Q, 1)
        for fn in norms:                     # final phase's norms
            fn()
        if NPH >= 8:
            proj_half(NQ - 1, 0, tail=True)
            proj_half(NQ - 1, 1, tail=True)
        ctx.close()
    return nc


_NC_CACHE = None


def _get_nc():
    global _NC_CACHE
    if _NC_CACHE is None:
        _NC_CACHE = build_nc()
    return _NC_CACHE


_EXEC_CACHE = None


def _get_executor():
    """Build + jit the SPMD executable once; reuse across kernel() calls."""
    global _EXEC_CACHE
    if _EXEC_CACHE is not None:
        return _EXEC_CACHE
    import jax
    from jax.sharding import Mesh, PartitionSpec
    from jax.experimental.shard_map import shard_map
    from concourse import bass2jax as b2j

    nc = _get_nc()
    b2j.install_neuronx_cc_hook()
    assert nc.dbg_addr is None
    partition_name = (
        nc.partition_id_tensor.name if nc.partition_id_tensor is not None else None
    )

    in_names, out_names, out_avals = [], [], []
    for alloc in nc.m.functions[0].allocations:
        if not isinstance(alloc, mybir.MemoryLocationSet):
            continue
        name = alloc.memorylocations[0].name
        if alloc.kind == "ExternalInput":
            if name != partition_name:
                in_names.append(name)
        elif alloc.kind == "ExternalOutput":
            out_names.append(name)
            out_avals.append(
                jax.core.ShapedArray(
                    tuple(alloc.tensor_shape), mybir.dt.np(alloc.dtype)
                )
            )
    n_params = len(in_names)
    n_outs = len(out_avals)
    all_names = in_names + out_names
    if partition_name is not None:
        all_names = all_names + [partition_name]

    def _body(*args):
        operands = list(args)
        if partition_name is not None:
            operands.append(b2j.partition_id_tensor())
        outs = b2j._bass_exec_p.bind(
            *operands,
            out_avals=tuple(out_avals),
            in_names=tuple(all_names),
            out_names=tuple(out_names),
            lowering_input_output_aliases=(),
            sim_require_finite=True,
            sim_require_nnan=True,
            nc=nc,
        )
        return tuple(outs)

    devices = jax.devices()[:NCORES]
    mesh = Mesh(np.asarray(devices), ("core",))
    donate = tuple(range(n_params, n_params + n_outs))
    sharded = jax.jit(
        shard_map(
            _body,
            mesh=mesh,
            in_specs=(PartitionSpec("core"),) * (n_params + n_outs),
            out_specs=(PartitionSpec("core"),) * n_outs,
            check_rep=False,
        ),
        donate_argnums=donate,
        keep_unused=True,
    )
    import jax.numpy as jnp

    zero_shardings = [
        jax.sharding.NamedSharding(mesh, PartitionSpec("core"))
    ] * n_outs

    @jax.jit
    def _make_zeros():
        return tuple(
            jax.lax.with_sharding_constraint(
                jnp.zeros((NCORES * a.shape[0], *a.shape[1:]), a.dtype), sh
            )
            for a, sh in zip(out_avals, zero_shardings)
        )

    _EXEC_CACHE = {
        "sharded": sharded,
        "make_zeros": _make_zeros,
        "in_names": in_names,
        "out_names": out_names,
        "out_avals": out_avals,
    }
    return _EXEC_CACHE


def _run_spmd(in_maps):
    ex = _get_executor()
    concat_in = [
        np.concatenate([np.asarray(m[name]) for m in in_maps], axis=0)
        for name in ex["in_names"]
    ]
    concat_zeros = ex["make_zeros"]()
    out_arrs = ex["sharded"](*concat_in, *concat_zeros)
    results = []
    for c in range(NCORES):
        results.append({
            name: np.asarray(out_arrs[i]).reshape(
                NCORES, *ex["out_avals"][i].shape
            )[c]
            for i, name in enumerate(ex["out_names"])
        })
    return results


def _shard_inputs(x, Wq, Wk, Wv, Wo):
    scale = np.float32(1.0 / np.sqrt(HD))
    in_maps = []
    xT_b = [np.ascontiguousarray(x[b].T).astype(np.float16) for b in range(B)]
    for c in range(NCORES):
        b, g = divmod(c, GROUPS)
        sl = slice(g * E, (g + 1) * E)
        wq = (Wq[sl, :] * scale).T.astype(np.float16)   # [D, 256]
        wk = Wk[sl, :].T.astype(np.float16)
        wv = Wv[sl, :].T.astype(np.float16)
        # k pair0 | k pair1 | q pair0 | q pair1 | v
        wqkv = np.concatenate([wk, wq, wv], axis=1)
        in_maps.append({
            "xT": xT_b[b],
            "wqkv": np.ascontiguousarray(wqkv),
            "woT": np.ascontiguousarray(Wo[:, sl].T).astype(np.float16),
        })
    return in_maps


_FAST_PATH_OK = True


def kernel(x, Wq, Wk, Wv, Wo, bo):
    global _FAST_PATH_OK
    x = np.asarray(x, dtype=np.float32)
    in_maps = _shard_inputs(
        x,
        np.asarray(Wq, dtype=np.float32),
        np.asarray(Wk, dtype=np.float32),
        np.asarray(Wv, dtype=np.float32),
        np.asarray(Wo, dtype=np.float32),
    )
    results = None
    if _FAST_PATH_OK:
        try:
            results = _run_spmd(in_maps)
        except Exception:
            _FAST_PATH_OK = False
    if results is None:
        results = run_bass_kernel_spmd(
            _get_nc(), in_maps, list(range(NCORES))
        ).results
    bo = np.asarray(bo, dtype=np.float32)
    out = np.empty((B, S, D), dtype=np.float32)
    for b in range(B):
        acc = np.zeros((S, D), dtype=np.float64)
        for g in range(GROUPS):
            acc += results[b * GROUPS + g]["out_partial"]
        out[b] = (acc + bo.astype(np.float64)).astype(np.float32)
    return out


# revision 78
# speedup vs baseline: 1.0095x; 1.0095x over previous
"""Multi-head attention (B=2, S=2048, D=1024, H=16) on 8 Trainium2 NeuronCores.

Sharding: core c handles batch b = c//4 and head group g = c%4 (4 heads = 2
head-pairs, 256 model dims).  Each core computes q/k/v projections for its
heads, attention, and a row-parallel partial output projection; the host sums
the 4 partials per batch and adds the bias.

All tensors are fp16 (1 cycle/row on the PE at any moving width, vs f32r's
>=256 requirement), halving DMA and SBUF vs f32.  Layouts:
  xT   [d, s]        (host pre-transposed)
  qT/kT [e, s]       per head-pair tile (head 2m at partitions 0-63, 2m+1 at
                     64-127) so the scores contraction dim (hd=64) sits on
                     partitions
  v    [ks, ksb, h, hd+1]  with a trailing ones column: the PV matmul's 65th
                     output column is the softmax denominator
  scores [ks, qs]    per head; exp'd tiles pt feed PV as the STATIONARY
                     operand (moving = v, N=65) so PV charges N=65/row
                     instead of N=qs -- half the cost of the [e, qs]
                     orientation
  ctx  [qs, e]       normalized via a per-partition reciprocal multiply (the
                     denominator lands on the qs partition axis -- no DRAM
                     broadcast round-trip), then block-transposed to [e, qs]
                     by a DMA xbar transpose for the output projection.

Softmax needs no max-subtraction (scores ~ N(0,1)).  exp splits across
engines per tile: ACT runs native Exp on cols [0:416); DVE computes cols
[416:512) with a Schraudolph bit-trick (y = int16(x*1024/ln2 + 15330)
bitcast to fp16, ~1.8% RMS sawtooth on 18.75% of weights) so the softmax
never throttles the PE.

PSUM (8 banks): shared [128,512] ring "gp" (2) for qk/v/proj groups, scores
pair ring [128,2,512]x2 (4), ctx accumulators [128,2,130]x2 (2).

Cost-model makespan per core target: ~150 us (PE-bound: QKV 41us + scores
54.6us + PV 27.7us + proj 13.7us at 2.4GHz warm).
"""

import os
import sys

import numpy as np

for _p in ("/opt/trn_rl_repo", "/root/.axon_site/_ro/trn_rl_repo"):
    if os.path.isdir(_p) and _p not in sys.path:
        sys.path.insert(0, _p)

import bass_rust
import concourse.bass as bass
import concourse.mybir as mybir
import concourse.tile as tile
from concourse.bass_utils import run_bass_kernel_spmd
from concourse.vector_clock import ScopedClock, VectorClock
from contextlib import ExitStack

F32 = mybir.dt.float32
F16 = mybir.dt.float16
I16 = mybir.dt.int16
EXP = mybir.ActivationFunctionType.Exp
MULT = mybir.AluOpType.mult
ADD = mybir.AluOpType.add

B = 2
S = 2048
D = 1024
H = 16
HD = 64
NCORES = 8
GROUPS = 4          # head groups (cores per batch)
HG = H // GROUPS    # heads per core = 4
E = HG * HD         # head dims per core = 256
KT = D // 128       # contraction tiles over model dim = 8
SB = S // 128       # 128-row s blocks = 16
NQ = 4              # qs quarters (512 wide)

# exp engine split within each [128, 2, 512] scores tile: the scores MMs
# write qs-128-blocks rotated by ksb, so ACT always runs native Exp on the
# contiguous sc cols [0:384) (one instruction) and DVE runs a Schraudolph
# bit-trick on [384:512) -- every qs block gets the approximation on exactly
# 4/16 of its ks blocks.
SCH_A = float(1024.0 / np.log(2.0))
SCH_B = 15360.0 - 30.0

_carrier_counter = [0]


def _split_multi_waits(ordered):
    """This walrus build allows one sync wait per instruction; Tile's wait
    assignment can attach several.  Hoist extras onto same-engine InstNoOp
    carriers placed immediately before the instruction."""
    for bb_name, insts in ordered.items():
        new_list = []
        for inst in insts:
            si = inst.sync_info
            waits = list(si.on_wait) if si is not None else []
            if len(waits) > 1:
                for w in waits[:-1]:
                    _carrier_counter[0] += 1
                    carrier = mybir.InstNoOp(
                        name=f"I-waitc-{_carrier_counter[0]}", ins=[], outs=[]
                    )
                    carrier.engine = inst.engine
                    carrier.sync_info = bass_rust.SyncInfo(on_wait=[w], on_update=[])
                    new_list.append(carrier)
                inst.sync_info = bass_rust.SyncInfo(
                    on_wait=[waits[-1]],
                    on_update=list(si.on_update) if si is not None else [],
                )
            new_list.append(inst)
        ordered[bb_name] = new_list


class _TileContext(tile.TileContext):
    """TileContext adapted to the one-sync-wait-per-instruction walrus."""

    def _lower_ordered_insts(self, ordered):
        _split_multi_waits(ordered)
        return super()._lower_ordered_insts(ordered)

    def _drain_and_barrier(self, tick_clock, wait_clock):
        gc = tick_clock.global_clock
        for proc in range(len(gc)):
            if gc[proc] <= 0:
                continue
            cur = VectorClock([0 if i == proc else gc[i] for i in range(len(gc))])
            nop = self.nc.sync.nop()
            wait_clock.add_sem_waits(
                nop.ins, ScopedClock({None: gc}), ScopedClock({None: cur})
            )
        drain_inst = self.nc.sync.drain()
        wait_clock.add_sem_waits(
            drain_inst.ins, ScopedClock({None: gc}), ScopedClock({None: gc.copy()})
        )
        self.nc.all_engine_barrier()
        assert self.sems is not None
        popped = self.nc._tile_sem_poison_stack.pop()
        assert popped is self._sem_poison
        self.nc.clear_and_free_semaphores(list(self.sems.allocated().values()))
        self.nc.all_engine_barrier()


def build_nc(reps=1):
    nc = bass.Bass()
    xT = nc.declare_dram_parameter("xT", [D, S], F16, isOutput=False)
    # wqkv cols: k pair0 | k pair1 | q pair0 | q pair1 | v (4 heads x 64)
    wqkv = nc.declare_dram_parameter("wqkv", [D, 768], F16, isOutput=False)
    woT = nc.declare_dram_parameter("woT", [E, D], F16, isOutput=False)
    out = nc.declare_dram_parameter("out_partial", [S, D], F16, isOutput=True)

    with _TileContext(nc) as tc, ExitStack() as outer:
      for _rep in range(reps):
        ctx = outer.enter_context(ExitStack())
        act_pool = ctx.enter_context(tc.tile_pool(name="acts", bufs=1))
        x_sb = act_pool.tile([128, KT, S], F16, tag="x")
        w_sb = act_pool.tile([128, KT, 768], F16, tag="w")
        wo_sb = act_pool.tile([128, 2, D], F16, tag="wo")
        qT = [act_pool.tile([128, S], F16, tag=f"qT{m}", name=f"qT{m}") for m in range(2)]
        # kT is stored zero-padded per head (version r has head r's rows,
        # zeros elsewhere) so the scores matmuls contract K=128 at tile
        # (0,0): mixing fp16 matmuls across PE row-tile offsets 0/64 hangs
        # the hardware.  The paired qT stays as the moving operand -- the
        # stationary zeros cancel the other head's rows.
        kT = [act_pool.tile([128, 2, S], F16, tag=f"kT{m}", name=f"kT{m}") for m in range(2)]
        v_sb = act_pool.tile([128, SB, HG, HD + 1], F16, tag="v")
        ctx_sb = [act_pool.tile([128, SB, 128], F16, tag=f"cs{m}", name=f"cs{m}") for m in range(2)]
        ctxT = [act_pool.tile([128, SB, 128], F16, tag=f"ct{m}", name=f"ct{m}") for m in range(2)]

        # ---- input DMAs, ordered for earliest PE start: kT0 weights, x
        # chunk 0, then the rest ----
        def dma_w(c0, c1):
            nc.sync.dma_start(
                w_sb[:, :, c0:c1],
                wqkv[:, c0:c1].rearrange("(k p) e -> p k e", p=128),
            )

        def dma_x(c, s0=0, s1=512):
            nc.sync.dma_start(
                x_sb[:, :, c * 512 + s0:c * 512 + s1],
                xT[:, c * 512 + s0:c * 512 + s1].rearrange(
                    "(k p) s -> p k s", p=128
                ),
            )

        # k-pair-0 weights split by kt halves so the first accumulation
        # group's kt0-3 matmuls start ~1us earlier
        nc.sync.dma_start(
            w_sb[:, 0:4, 0:128],
            wqkv[0:512, 0:128].rearrange("(k p) e -> p k e", p=128),
        )
        nc.sync.dma_start(
            w_sb[:, 4:8, 0:128],
            wqkv[512:1024, 0:128].rearrange("(k p) e -> p k e", p=128),
        )
        dma_x(0, 0, 256)
        dma_x(0, 256, 512)
        dma_w(256, 384)      # q pair 0
        dma_w(512, 768)      # v
        dma_w(128, 256)      # k pair 1
        dma_x(1)
        dma_x(2)
        dma_w(384, 512)      # q pair 1
        dma_x(3)
        nc.sync.dma_start(
            wo_sb[:, :, :], woT[:, :].rearrange("(m p) e -> p m e", p=128)
        )
        # ones column for the softmax-denominator rows of v
        nc.gpsimd.memset(v_sb[:, :, :, HD], 1.0)
        # zero the padded halves of the kT versions (Pool; SBUF only)
        for m in range(2):
            for r in range(2):
                nc.gpsimd.memset(kT[m][64 * (1 - r):64 * (1 - r) + 64, r, :], 0.0)

        # ---- shared PSUM rings (8 banks: sc 2x2, sc_d 1, cx 2, gp 1) ----
        ps_gp = ctx.enter_context(tc.tile_pool(name="ps_gp", bufs=1, space="PSUM"))
        ps_sc = ctx.enter_context(tc.tile_pool(name="ps_sc", bufs=2, space="PSUM"))
        ps_scd = ctx.enter_context(tc.tile_pool(name="ps_scd", bufs=1, space="PSUM"))
        ps_cx = ctx.enter_context(tc.tile_pool(name="ps_cx", bufs=1, space="PSUM"))
        pt_pool = ctx.enter_context(tc.tile_pool(name="pt", bufs=8))
        ptd_pool = ctx.enter_context(tc.tile_pool(name="ptd", bufs=8))
        rc_pool = ctx.enter_context(tc.tile_pool(name="rc", bufs=4))
        st_pool = ctx.enter_context(tc.tile_pool(name="st", bufs=6))

        # GPSIMD cannot touch PSUM, so every PSUM exit goes through ACT or
        # DVE: ACT takes the early copies (its exp chain hasn't started),
        # DVE the rest (interleaved between Schraudolphs)
        def copy(dst, src, eng=None):
            eng = eng or nc.vector
            if eng is nc.scalar:
                eng.copy(dst, src)
            else:
                eng.tensor_copy(dst, src)

        pre_i = [0]

        def pre_ps():
            # pre-attention groups rotate through the idle sc/scd-ring banks
            # as well as gp, a 4-deep ring that hides the copy WAR chain
            pre_i[0] += 1
            ph = pre_i[0] % 4
            if ph == 0:
                return ps_gp.tile([128, 512], F32, tag="gp", name="qk")
            if ph == 3:
                return ps_scd.tile([128, 512], F32, tag="scd", name="qk")
            t = ps_sc.tile([128, 2, 384], F32, tag="sc", name="qk")
            return t.rearrange("p a b -> p (a b)")[:, 0:512]

        def qk_group(w_off, dst, c, width=512, off=0, eng=None, pad_k=False,
                     pre=False):
            sl = slice(c * 512 + off, c * 512 + off + width)
            ps = pre_ps() if pre else ps_gp.tile(
                [128, 512], F32, tag="gp", name="qk")
            for k in range(KT):
                nc.tensor.matmul(
                    ps[:, 0:width],
                    w_sb[:, k, w_off:w_off + 128],
                    x_sb[:, k, sl],
                    start=(k == 0),
                    stop=(k == KT - 1),
                )
            if pad_k:
                # split the pair rows into the two zero-padded kT versions
                copy(dst[0:64, 0, sl], ps[0:64, 0:width], eng)
                copy(dst[64:128, 1, sl], ps[64:128, 0:width], eng)
            else:
                copy(dst[:, sl], ps[:, 0:width], eng)

        def v_group(sb):
            ps = pre_ps()
            for k in range(KT):
                nc.tensor.matmul(
                    ps[:, 0:256],
                    x_sb[:, k, sb * 128:(sb + 1) * 128],
                    w_sb[:, k, 512:768],
                    start=(k == 0),
                    stop=(k == KT - 1),
                )
            copy(
                v_sb[:, sb, :, 0:HD],
                ps[:, 0:256].rearrange("p (h e) -> p h e", h=HG),
                nc.scalar,
            )

        def attention_phase(m, Q, carried_norms=()):
            # scores qs-block q lands at column block w = (q + ksb) % 4.
            # Blocks 0-2 go to the sc tile (read ONLY by ACT's native exp);
            # block 3 goes to the separate 1-bank sc_d tile (read ONLY by
            # DVE's Schraudolph): Tile serializes same-PSUM-tile readers, so
            # the reader split keeps ACT's exp chain free of the DVE.  The
            # rotation gives every qs column the approximation on exactly
            # 4/16 of its ks blocks.
            cx = [
                ps_cx.tile([128, 2, 2 * (HD + 1)], F32, tag=f"cx{j}", name=f"cx{j}")
                for j in range(2)
            ]
            # software-pipelined by 2: the static in-order PE queue must see
            # scores(t) BEFORE PV(t-2); otherwise a PV stalled on its exp
            # blocks the independent scores behind it and the PE idles for
            # most of every ACT instruction
            pts = {}
            for t in range(SB + 2):
                # the previous phase's normalize ops slot into DVE's per-
                # iteration slack here, never blocking a Schraudolph long
                if t - 1 < len(carried_norms) and t >= 1:
                    carried_norms[t - 1]()
                if t < SB:
                    ksb = t
                    sc = ps_sc.tile([128, 2, 384], F32, tag="sc", name="sc")
                    scd = ps_scd.tile([128, 2, 128], F32, tag="scd", name="scd")
                    for w in (0, 1, 2, 3):
                        q = (w - ksb) % 4
                        for r in range(2):
                            out_ap = (
                                scd[:, r, :] if w == 3
                                else sc[:, r, w * 128:(w + 1) * 128]
                            )
                            nc.tensor.matmul(
                                out_ap,
                                kT[m][:, r, ksb * 128:(ksb + 1) * 128],
                                qT[m][:, Q * 512 + q * 128:Q * 512 + (q + 1) * 128],
                                start=True,
                                stop=True,
                            )
                    pt = pt_pool.tile([128, 2, 384], F16, tag="pt", name="pt")
                    ptd = ptd_pool.tile([128, 2, 128], F16, tag="ptd", name="ptd")
                    nc.scalar.activation(pt[:, :, :], sc[:, :, :], EXP)
                    if os.environ.get("KDBG_NO_SCH"):
                        nc.vector.tensor_copy(ptd[:, :, :], scd[:, :, :])
                    else:
                        nc.vector.tensor_scalar(
                            ptd[:, :, :].bitcast(I16),
                            scd[:, :, :],
                            SCH_A,
                            SCH_B,
                            MULT,
                            ADD,
                        )
                    pts[ksb] = (pt, ptd)
                if t >= 2 and not os.environ.get("KDBG_NO_PV"):
                    ksb = t - 2
                    pt, ptd = pts.pop(ksb)
                    # one accumulation group per cx tile (2KB zero region):
                    # start only on the tile's first MM, stop on its last
                    for j in range(2):
                        for qq in range(2):
                            for r in range(2):
                                w = (j * 2 + qq + ksb) % 4
                                stat = (
                                    ptd[:, r, :] if w == 3
                                    else pt[:, r, w * 128:(w + 1) * 128]
                                )
                                nc.tensor.matmul(
                                    cx[j][:, qq, r * (HD + 1):(r + 1) * (HD + 1)],
                                    stat,
                                    v_sb[:, ksb, 2 * m + r, :],
                                    start=(ksb == 0 and qq == 0 and r == 0),
                                    stop=(ksb == SB - 1 and qq == 1 and r == 1),
                                )
            # normalize: ctx = ctxU * (1/den); den is PV output col 64 per
            # head (a per-partition scalar on the qs axis), then xbar-
            # transpose each 2-qsb block to [e, qs] for the projection.
            # Returned as closures the NEXT phase emits into its DVE slack.
            def norm_part(j, qq, with_recip, with_transpose, rc_box={}):
                def emit():
                    if os.environ.get("KDBG_NO_NORM"):
                        return
                    if with_recip:
                        rc = rc_pool.tile([128, 2, 2], F32, tag="rc", name="rc")
                        nc.vector.reciprocal(
                            rc[:], cx[j][:, :, HD:2 * (HD + 1):HD + 1]
                        )
                        rc_box[j] = rc
                    rc = rc_box[j]
                    qsb = Q * 4 + j * 2 + qq
                    nc.vector.tensor_mul(
                        ctx_sb[m][:, qsb, :].rearrange("p (h e) -> p h e", h=2),
                        cx[j][:, qq, :].rearrange(
                            "p (h e) -> p h e", e=HD + 1
                        )[:, :, 0:HD],
                        rc[:, qq, :].broadcast_to([128, 2, HD]),
                    )
                    if with_transpose:
                        if os.environ.get("KDBG_NO_TRANSPOSE"):
                            nc.sync.dma_start(
                                ctxT[m][:, Q * 4 + 2 * j:Q * 4 + 2 * j + 2, :],
                                ctx_sb[m][:, Q * 4 + 2 * j:Q * 4 + 2 * j + 2, :],
                            )
                        else:
                            nc.sync.dma_start_transpose(
                                ctxT[m][:, Q * 4 + 2 * j:Q * 4 + 2 * j + 2, :],
                                ctx_sb[m][:, Q * 4 + 2 * j:Q * 4 + 2 * j + 2, :],
                            )
                return emit

            box = {}
            return [
                norm_part(0, 0, True, False, box),
                norm_part(0, 1, False, True, box),
                norm_part(1, 0, True, False, box),
                norm_part(1, 1, False, True, box),
            ]

        def proj_half(Q, j, tail=False):
            for qsb in range(Q * 4 + 2 * j, Q * 4 + 2 * j + 2):
                stage = st_pool.tile([128, D], F16, tag="st", name="st")
                for nb in range(2):
                    # the final quarter's projections use the freed cx banks
                    # so the tail is double-buffered despite gp bufs=1
                    pso = (
                        ps_cx.tile([128, 512], F32, tag=f"cx{nb}", name="o")
                        if tail
                        else ps_gp.tile([128, 512], F32, tag="gp", name="o")
                    )
                    for m in range(2):
                        nc.tensor.matmul(
                            pso[:],
                            ctxT[m][:, qsb, :],
                            wo_sb[:, m, nb * 512:(nb + 1) * 512],
                            start=(m == 0),
                            stop=(m == 1),
                        )
                    copy(stage[:, nb * 512:(nb + 1) * 512], pso[:],
                         (nc.scalar if nb == 0 else nc.vector)
                         if tail else nc.vector)
                    if tail:
                        nc.sync.dma_start(
                            out[qsb * 128:(qsb + 1) * 128,
                                nb * 512:(nb + 1) * 512],
                            stage[:, nb * 512:(nb + 1) * 512],
                        )
                if not tail:
                    nc.sync.dma_start(out[qsb * 128:(qsb + 1) * 128, :], stage[:])

        # ---- emission order = scheduler priority: the minimum needed for
        # attention (m0, Q0) first, then the deferred q projections and
        # output projections as PE gap filler while exp chains bound the
        # attention phases ----
        # chunk-major so no x-gated group ever sits ahead of ready work;
        # the pre-attention copies run on ACT (its exp chain is idle there)
        qk_group(0, kT[0], 0, 256, eng=nc.scalar, pad_k=True, pre=True)
        qk_group(0, kT[0], 0, 256, 256, eng=nc.scalar, pad_k=True, pre=True)
        qk_group(256, qT[0], 0, 256, eng=nc.scalar, pre=True)
        qk_group(256, qT[0], 0, 256, 256, eng=nc.scalar, pre=True)
        qk_group(128, kT[1], 0, eng=nc.scalar, pad_k=True, pre=True)
        qk_group(384, qT[1], 0, eng=nc.scalar, pre=True)
        for sb in range(4):
            v_group(sb)
        for c in range(1, NQ):
            qk_group(0, kT[0], c, eng=nc.scalar, pad_k=True, pre=True)
            for sb in range(c * 4, c * 4 + 4):
                v_group(sb)
        NPH = int(os.environ.get("KDBG_NPHASES", "8"))
        phase_list = [(0, 0), (1, 0), (0, 1), (1, 1), (0, 2), (1, 2),
                      (0, 3), (1, 3)][:NPH]
        pi = [0]

        def next_phase(norms):
            if pi[0] >= len(phase_list):
                return norms
            m, Q = phase_list[pi[0]]
            pi[0] += 1
            return attention_phase(m, Q, norms)

        norms = next_phase(())                     # (0,0)
        for c in range(1, NQ):
            qk_group(128, kT[1], c, pad_k=True)
        qk_group(256, qT[0], 1)
        norms = next_phase(norms)                  # (1,0)
        qk_group(384, qT[1], 1)
        norms = next_phase(norms)                  # (0,1)
        if NPH >= 3:
            proj_half(0, 0)
        qk_group(256, qT[0], 2)
        if NPH >= 3:
            proj_half(0, 1)
        for Q in range(1, NQ):
            norms = next_phase(norms)              # (1,Q)
            if Q < NQ - 1:
                qk_group(384, qT[1], Q + 1)
                norms = next_phase(norms)          # (0,Q+1)
                if NPH >= 2 * Q + 3:
                    proj_half(Q, 0)
                if Q < NQ - 2:
                    qk_group(256, qT[0], Q + 2)
                if NPH >= 2 * Q + 3:
                    proj_half(Q, 1)
        for fn in norms:                     # final phase's norms
            fn()
        if NPH >= 8:
            proj_half(NQ - 1, 0, tail=True)
            proj_half(NQ - 1, 1, tail=True)
        ctx.close()
    return nc


_NC_CACHE = None


def _get_nc():
    global _NC_CACHE
    if _NC_CACHE is None:
        _NC_CACHE = build_nc()
    return _NC_CACHE


_EXEC_CACHE = None


def _get_executor():
    """Build + jit the SPMD executable once; reuse across kernel() calls."""
    global _EXEC_CACHE
    if _EXEC_CACHE is not None:
        return _EXEC_CACHE
    import jax
    from jax.sharding import Mesh, PartitionSpec
    from jax.experimental.shard_map import shard_map
    from concourse import bass2jax as b2j

    nc = _get_nc()
    b2j.install_neuronx_cc_hook()
    assert nc.dbg_addr is None
    partition_name = (
        nc.partition_id_tensor.name if nc.partition_id_tensor is not None else None
    )

    in_names, out_names, out_avals = [], [], []
    for alloc in nc.m.functions[0].allocations:
        if not isinstance(alloc, mybir.MemoryLocationSet):
            continue
        name = alloc.memorylocations[0].name
        if alloc.kind == "ExternalInput":
            if name != partition_name:
                in_names.append(name)
        elif alloc.kind == "ExternalOutput":
            out_names.append(name)
            out_avals.append(
                jax.core.ShapedArray(
                    tuple(alloc.tensor_shape), mybir.dt.np(alloc.dtype)
                )
            )
    n_params = len(in_names)
    n_outs = len(out_avals)
    all_names = in_names + out_names
    if partition_name is not None:
        all_names = all_names + [partition_name]

    def _body(*args):
        operands = list(args)
        if partition_name is not None:
            operands.append(b2j.partition_id_tensor())
        outs = b2j._bass_exec_p.bind(
            *operands,
            out_avals=tuple(out_avals),
            in_names=tuple(all_names),
            out_names=tuple(out_names),
            lowering_input_output_aliases=(),
            sim_require_finite=True,
            sim_require_nnan=True,
            nc=nc,
        )
        return tuple(outs)

    devices = jax.devices()[:NCORES]
    mesh = Mesh(np.asarray(devices), ("core",))
    donate = tuple(range(n_params, n_params + n_outs))
    sharded = jax.jit(
        shard_map(
            _body,
            mesh=mesh,
            in_specs=(PartitionSpec("core"),) * (n_params + n_outs),
            out_specs=(PartitionSpec("core"),) * n_outs,
            check_rep=False,
        ),
        donate_argnums=donate,
        keep_unused=True,
    )
    import jax.numpy as jnp

    zero_shardings = [
        jax.sharding.NamedSharding(mesh, PartitionSpec("core"))
    ] * n_outs

    @jax.jit
    def _make_zeros():
        return tuple(
            jax.lax.with_sharding_constraint(
                jnp.zeros((NCORES * a.shape[0], *a.shape[1:]), a.dtype), sh
            )
            for a, sh in zip(out_avals, zero_shardings)
        )

    _EXEC_CACHE = {
        "sharded": sharded,
        "make_zeros": _make_zeros,
        "in_names": in_names,
        "out_names": out_names,
        "out_avals": out_avals,
    }
    return _EXEC_CACHE


def _run_spmd(in_maps):
    ex = _get_executor()
    concat_in = [
        np.concatenate([np.asarray(m[name]) for m in in_maps], axis=0)
        for name in ex["in_names"]
    ]
    concat_zeros = ex["make_zeros"]()
    out_arrs = ex["sharded"](*concat_in, *concat_zeros)
    results = []
    for c in range(NCORES):
        results.append({
            name: np.asarray(out_arrs[i]).reshape(
                NCORES, *ex["out_avals"][i].shape
            )[c]
            for i, name in enumerate(ex["out_names"])
        })
    return results


def _shard_inputs(x, Wq, Wk, Wv, Wo):
    scale = np.float32(1.0 / np.sqrt(HD))
    in_maps = []
    xT_b = [np.ascontiguousarray(x[b].T).astype(np.float16) for b in range(B)]
    for c in range(NCORES):
        b, g = divmod(c, GROUPS)
        sl = slice(g * E, (g + 1) * E)
        wq = (Wq[sl, :] * scale).T.astype(np.float16)   # [D, 256]
        wk = Wk[sl, :].T.astype(np.float16)
        wv = Wv[sl, :].T.astype(np.float16)
        # k pair0 | k pair1 | q pair0 | q pair1 | v
        wqkv = np.concatenate([wk, wq, wv], axis=1)
        in_maps.append({
            "xT": xT_b[b],
            "wqkv": np.ascontiguousarray(wqkv),
            "woT": np.ascontiguousarray(Wo[:, sl].T).astype(np.float16),
        })
    return in_maps


_FAST_PATH_OK = True


def kernel(x, Wq, Wk, Wv, Wo, bo):
    global _FAST_PATH_OK
    x = np.asarray(x, dtype=np.float32)
    in_maps = _shard_inputs(
        x,
        np.asarray(Wq, dtype=np.float32),
        np.asarray(Wk, dtype=np.float32),
        np.asarray(Wv, dtype=np.float32),
        np.asarray(Wo, dtype=np.float32),
    )
    results = None
    if _FAST_PATH_OK:
        try:
            results = _run_spmd(in_maps)
        except Exception:
            _FAST_PATH_OK = False
    if results is None:
        results = run_bass_kernel_spmd(
            _get_nc(), in_maps, list(range(NCORES))
        ).results
    bo = np.asarray(bo, dtype=np.float32)
    out = np.empty((B, S, D), dtype=np.float32)
    for b in range(B):
        acc = np.zeros((S, D), dtype=np.float64)
        for g in range(GROUPS):
            acc += results[b * GROUPS + g]["out_partial"]
        out[b] = (acc + bo.astype(np.float64)).astype(np.float32)
    return out


# revision 79
# speedup vs baseline: 1.0175x; 1.0079x over previous
"""Multi-head attention (B=2, S=2048, D=1024, H=16) on 8 Trainium2 NeuronCores.

Sharding: core c handles batch b = c//4 and head group g = c%4 (4 heads = 2
head-pairs, 256 model dims).  Each core computes q/k/v projections for its
heads, attention, and a row-parallel partial output projection; the host sums
the 4 partials per batch and adds the bias.

All tensors are fp16 (1 cycle/row on the PE at any moving width, vs f32r's
>=256 requirement), halving DMA and SBUF vs f32.  Layouts:
  xT   [d, s]        (host pre-transposed)
  qT/kT [e, s]       per head-pair tile (head 2m at partitions 0-63, 2m+1 at
                     64-127) so the scores contraction dim (hd=64) sits on
                     partitions
  v    [ks, ksb, h, hd+1]  with a trailing ones column: the PV matmul's 65th
                     output column is the softmax denominator
  scores [ks, qs]    per head; exp'd tiles pt feed PV as the STATIONARY
                     operand (moving = v, N=65) so PV charges N=65/row
                     instead of N=qs -- half the cost of the [e, qs]
                     orientation
  ctx  [qs, e]       normalized via a per-partition reciprocal multiply (the
                     denominator lands on the qs partition axis -- no DRAM
                     broadcast round-trip), then block-transposed to [e, qs]
                     by a DMA xbar transpose for the output projection.

Softmax needs no max-subtraction (scores ~ N(0,1)).  exp splits across
engines per tile: ACT runs native Exp on cols [0:416); DVE computes cols
[416:512) with a Schraudolph bit-trick (y = int16(x*1024/ln2 + 15330)
bitcast to fp16, ~1.8% RMS sawtooth on 18.75% of weights) so the softmax
never throttles the PE.

PSUM (8 banks): shared [128,512] ring "gp" (2) for qk/v/proj groups, scores
pair ring [128,2,512]x2 (4), ctx accumulators [128,2,130]x2 (2).

Cost-model makespan per core target: ~150 us (PE-bound: QKV 41us + scores
54.6us + PV 27.7us + proj 13.7us at 2.4GHz warm).
"""

import os
import sys

import numpy as np

for _p in ("/opt/trn_rl_repo", "/root/.axon_site/_ro/trn_rl_repo"):
    if os.path.isdir(_p) and _p not in sys.path:
        sys.path.insert(0, _p)

import bass_rust
import concourse.bass as bass
import concourse.mybir as mybir
import concourse.tile as tile
from concourse.bass_utils import run_bass_kernel_spmd
from concourse.vector_clock import ScopedClock, VectorClock
from contextlib import ExitStack

F32 = mybir.dt.float32
F16 = mybir.dt.float16
I16 = mybir.dt.int16
EXP = mybir.ActivationFunctionType.Exp
MULT = mybir.AluOpType.mult
ADD = mybir.AluOpType.add

B = 2
S = 2048
D = 1024
H = 16
HD = 64
NCORES = 8
GROUPS = 4          # head groups (cores per batch)
HG = H // GROUPS    # heads per core = 4
E = HG * HD         # head dims per core = 256
KT = D // 128       # contraction tiles over model dim = 8
SB = S // 128       # 128-row s blocks = 16
NQ = 4              # qs quarters (512 wide)

# exp engine split within each [128, 2, 512] scores tile: the scores MMs
# write qs-128-blocks rotated by ksb, so ACT always runs native Exp on the
# contiguous sc cols [0:384) (one instruction) and DVE runs a Schraudolph
# bit-trick on [384:512) -- every qs block gets the approximation on exactly
# 4/16 of its ks blocks.
SCH_A = float(1024.0 / np.log(2.0))
SCH_B = 15360.0 - 30.0

_carrier_counter = [0]


def _split_multi_waits(ordered):
    """This walrus build allows one sync wait per instruction; Tile's wait
    assignment can attach several.  Hoist extras onto same-engine InstNoOp
    carriers placed immediately before the instruction."""
    for bb_name, insts in ordered.items():
        new_list = []
        for inst in insts:
            si = inst.sync_info
            waits = list(si.on_wait) if si is not None else []
            if len(waits) > 1:
                for w in waits[:-1]:
                    _carrier_counter[0] += 1
                    carrier = mybir.InstNoOp(
                        name=f"I-waitc-{_carrier_counter[0]}", ins=[], outs=[]
                    )
                    carrier.engine = inst.engine
                    carrier.sync_info = bass_rust.SyncInfo(on_wait=[w], on_update=[])
                    new_list.append(carrier)
                inst.sync_info = bass_rust.SyncInfo(
                    on_wait=[waits[-1]],
                    on_update=list(si.on_update) if si is not None else [],
                )
            new_list.append(inst)
        ordered[bb_name] = new_list


class _TileContext(tile.TileContext):
    """TileContext adapted to the one-sync-wait-per-instruction walrus."""

    def _lower_ordered_insts(self, ordered):
        _split_multi_waits(ordered)
        return super()._lower_ordered_insts(ordered)

    def _drain_and_barrier(self, tick_clock, wait_clock):
        gc = tick_clock.global_clock
        for proc in range(len(gc)):
            if gc[proc] <= 0:
                continue
            cur = VectorClock([0 if i == proc else gc[i] for i in range(len(gc))])
            nop = self.nc.sync.nop()
            wait_clock.add_sem_waits(
                nop.ins, ScopedClock({None: gc}), ScopedClock({None: cur})
            )
        drain_inst = self.nc.sync.drain()
        wait_clock.add_sem_waits(
            drain_inst.ins, ScopedClock({None: gc}), ScopedClock({None: gc.copy()})
        )
        self.nc.all_engine_barrier()
        assert self.sems is not None
        popped = self.nc._tile_sem_poison_stack.pop()
        assert popped is self._sem_poison
        self.nc.clear_and_free_semaphores(list(self.sems.allocated().values()))
        self.nc.all_engine_barrier()


def build_nc(reps=1):
    nc = bass.Bass()
    xT = nc.declare_dram_parameter("xT", [D, S], F16, isOutput=False)
    # wqkv cols: k pair0 | k pair1 | q pair0 | q pair1 | v (4 heads x 64)
    wqkv = nc.declare_dram_parameter("wqkv", [D, 768], F16, isOutput=False)
    woT = nc.declare_dram_parameter("woT", [E, D], F16, isOutput=False)
    ident = nc.declare_dram_parameter("ident", [128, 128], F16, isOutput=False)
    out = nc.declare_dram_parameter("out_partial", [S, D], F16, isOutput=True)

    with _TileContext(nc) as tc, ExitStack() as outer:
      for _rep in range(reps):
        ctx = outer.enter_context(ExitStack())
        act_pool = ctx.enter_context(tc.tile_pool(name="acts", bufs=1))
        x_sb = act_pool.tile([128, KT, S], F16, tag="x")
        w_sb = act_pool.tile([128, KT, 768], F16, tag="w")
        wo_sb = act_pool.tile([128, 2, D], F16, tag="wo")
        qT = [act_pool.tile([128, S], F16, tag=f"qT{m}", name=f"qT{m}") for m in range(2)]
        # kT is stored zero-padded per head (version r has head r's rows,
        # zeros elsewhere) so the scores matmuls contract K=128 at tile
        # (0,0): mixing fp16 matmuls across PE row-tile offsets 0/64 hangs
        # the hardware.  The paired qT stays as the moving operand -- the
        # stationary zeros cancel the other head's rows.
        kT = [act_pool.tile([128, 2, S], F16, tag=f"kT{m}", name=f"kT{m}") for m in range(2)]
        v_sb = act_pool.tile([128, SB, HG, HD + 1], F16, tag="v")
        ctx_sb = [act_pool.tile([128, SB, 128], F16, tag=f"cs{m}", name=f"cs{m}") for m in range(2)]
        ctxT = [act_pool.tile([128, SB, 128], F16, tag=f"ct{m}", name=f"ct{m}") for m in range(2)]

        # ---- input DMAs, ordered for earliest PE start: kT0 weights, x
        # chunk 0, then the rest ----
        def dma_w(c0, c1):
            nc.sync.dma_start(
                w_sb[:, :, c0:c1],
                wqkv[:, c0:c1].rearrange("(k p) e -> p k e", p=128),
            )

        def dma_x(c, s0=0, s1=512):
            nc.sync.dma_start(
                x_sb[:, :, c * 512 + s0:c * 512 + s1],
                xT[:, c * 512 + s0:c * 512 + s1].rearrange(
                    "(k p) s -> p k s", p=128
                ),
            )

        # k-pair-0 weights split by kt halves so the first accumulation
        # group's kt0-3 matmuls start ~1us earlier
        nc.sync.dma_start(
            w_sb[:, 0:4, 0:128],
            wqkv[0:512, 0:128].rearrange("(k p) e -> p k e", p=128),
        )
        nc.sync.dma_start(
            w_sb[:, 4:8, 0:128],
            wqkv[512:1024, 0:128].rearrange("(k p) e -> p k e", p=128),
        )
        dma_x(0, 0, 256)
        dma_x(0, 256, 512)
        dma_w(256, 384)      # q pair 0
        dma_w(512, 768)      # v
        dma_w(128, 256)      # k pair 1
        dma_x(1)
        dma_x(2)
        dma_w(384, 512)      # q pair 1
        dma_x(3)
        nc.sync.dma_start(
            wo_sb[:, :, :], woT[:, :].rearrange("(m p) e -> p m e", p=128)
        )
        id_sb = act_pool.tile([128, 128], F16, tag="ident")
        nc.sync.dma_start(id_sb[:], ident[:, :])
        # ones column for the softmax-denominator rows of v
        nc.gpsimd.memset(v_sb[:, :, :, HD], 1.0)
        # zero the padded halves of the kT versions (Pool; SBUF only)
        for m in range(2):
            for r in range(2):
                nc.gpsimd.memset(kT[m][64 * (1 - r):64 * (1 - r) + 64, r, :], 0.0)

        # ---- shared PSUM rings (8 banks: sc 2x2, sc_d 1, cx 2, gp 1) ----
        ps_gp = ctx.enter_context(tc.tile_pool(name="ps_gp", bufs=1, space="PSUM"))
        ps_sc = ctx.enter_context(tc.tile_pool(name="ps_sc", bufs=2, space="PSUM"))
        ps_scd = ctx.enter_context(tc.tile_pool(name="ps_scd", bufs=1, space="PSUM"))
        ps_cx = ctx.enter_context(tc.tile_pool(name="ps_cx", bufs=1, space="PSUM"))
        pt_pool = ctx.enter_context(tc.tile_pool(name="pt", bufs=8))
        ptd_pool = ctx.enter_context(tc.tile_pool(name="ptd", bufs=8))
        rc_pool = ctx.enter_context(tc.tile_pool(name="rc", bufs=4))
        st_pool = ctx.enter_context(tc.tile_pool(name="st", bufs=6))

        # GPSIMD cannot touch PSUM, so every PSUM exit goes through ACT or
        # DVE: ACT takes the early copies (its exp chain hasn't started),
        # DVE the rest (interleaved between Schraudolphs)
        def copy(dst, src, eng=None):
            eng = eng or nc.vector
            if eng is nc.scalar:
                eng.copy(dst, src)
            else:
                eng.tensor_copy(dst, src)

        pre_i = [0]

        def pre_ps():
            # pre-attention groups rotate through the idle sc/scd-ring banks
            # as well as gp, a 4-deep ring that hides the copy WAR chain
            pre_i[0] += 1
            ph = pre_i[0] % 4
            if ph == 0:
                return ps_gp.tile([128, 512], F32, tag="gp", name="qk")
            if ph == 3:
                return ps_scd.tile([128, 512], F32, tag="scd", name="qk")
            t = ps_sc.tile([128, 2, 384], F32, tag="sc", name="qk")
            return t.rearrange("p a b -> p (a b)")[:, 0:512]

        def qk_group(w_off, dst, c, width=512, off=0, eng=None, pad_k=False,
                     pre=False):
            sl = slice(c * 512 + off, c * 512 + off + width)
            ps = pre_ps() if pre else ps_gp.tile(
                [128, 512], F32, tag="gp", name="qk")
            for k in range(KT):
                nc.tensor.matmul(
                    ps[:, 0:width],
                    w_sb[:, k, w_off:w_off + 128],
                    x_sb[:, k, sl],
                    start=(k == 0),
                    stop=(k == KT - 1),
                )
            if pad_k:
                # split the pair rows into the two zero-padded kT versions
                copy(dst[0:64, 0, sl], ps[0:64, 0:width], eng)
                copy(dst[64:128, 1, sl], ps[64:128, 0:width], eng)
            else:
                copy(dst[:, sl], ps[:, 0:width], eng)

        def v_group(sb):
            ps = pre_ps()
            for k in range(KT):
                nc.tensor.matmul(
                    ps[:, 0:256],
                    x_sb[:, k, sb * 128:(sb + 1) * 128],
                    w_sb[:, k, 512:768],
                    start=(k == 0),
                    stop=(k == KT - 1),
                )
            copy(
                v_sb[:, sb, :, 0:HD],
                ps[:, 0:256].rearrange("p (h e) -> p h e", h=HG),
                nc.scalar,
            )

        def attention_phase(m, Q, carried_norms=(), pe_t=False):
            # scores qs-block q lands at column block w = (q + ksb) % 4.
            # Blocks 0-2 go to the sc tile (read ONLY by ACT's native exp);
            # block 3 goes to the separate 1-bank sc_d tile (read ONLY by
            # DVE's Schraudolph): Tile serializes same-PSUM-tile readers, so
            # the reader split keeps ACT's exp chain free of the DVE.  The
            # rotation gives every qs column the approximation on exactly
            # 4/16 of its ks blocks.
            cx = [
                ps_cx.tile([128, 2, 2 * (HD + 1)], F32, tag=f"cx{j}", name=f"cx{j}")
                for j in range(2)
            ]
            # software-pipelined by 2: the static in-order PE queue must see
            # scores(t) BEFORE PV(t-2); otherwise a PV stalled on its exp
            # blocks the independent scores behind it and the PE idles for
            # most of every ACT instruction
            pts = {}
            for t in range(SB + 2):
                # the previous phase's normalize ops slot into DVE's per-
                # iteration slack here, never blocking a Schraudolph long
                if t - 1 < len(carried_norms) and t >= 1:
                    carried_norms[t - 1]()
                if t < SB:
                    ksb = t
                    sc = ps_sc.tile([128, 2, 384], F32, tag="sc", name="sc")
                    scd = ps_scd.tile([128, 2, 128], F32, tag="scd", name="scd")
                    for w in (0, 1, 2, 3):
                        q = (w - ksb) % 4
                        for r in range(2):
                            out_ap = (
                                scd[:, r, :] if w == 3
                                else sc[:, r, w * 128:(w + 1) * 128]
                            )
                            nc.tensor.matmul(
                                out_ap,
                                kT[m][:, r, ksb * 128:(ksb + 1) * 128],
                                qT[m][:, Q * 512 + q * 128:Q * 512 + (q + 1) * 128],
                                start=True,
                                stop=True,
                            )
                    pt = pt_pool.tile([128, 2, 384], F16, tag="pt", name="pt")
                    ptd = ptd_pool.tile([128, 2, 128], F16, tag="ptd", name="ptd")
                    nc.scalar.activation(pt[:, :, :], sc[:, :, :], EXP)
                    if os.environ.get("KDBG_NO_SCH"):
                        nc.vector.tensor_copy(ptd[:, :, :], scd[:, :, :])
                    else:
                        nc.vector.tensor_scalar(
                            ptd[:, :, :].bitcast(I16),
                            scd[:, :, :],
                            SCH_A,
                            SCH_B,
                            MULT,
                            ADD,
                        )
                    pts[ksb] = (pt, ptd)
                if t >= 2 and not os.environ.get("KDBG_NO_PV"):
                    ksb = t - 2
                    pt, ptd = pts.pop(ksb)
                    # one accumulation group per cx tile (2KB zero region):
                    # start only on the tile's first MM, stop on its last
                    for j in range(2):
                        for qq in range(2):
                            for r in range(2):
                                w = (j * 2 + qq + ksb) % 4
                                stat = (
                                    ptd[:, r, :] if w == 3
                                    else pt[:, r, w * 128:(w + 1) * 128]
                                )
                                nc.tensor.matmul(
                                    cx[j][:, qq, r * (HD + 1):(r + 1) * (HD + 1)],
                                    stat,
                                    v_sb[:, ksb, 2 * m + r, :],
                                    start=(ksb == 0 and qq == 0 and r == 0),
                                    stop=(ksb == SB - 1 and qq == 1 and r == 1),
                                )
            # normalize: ctx = ctxU * (1/den); den is PV output col 64 per
            # head (a per-partition scalar on the qs axis), then xbar-
            # transpose each 2-qsb block to [e, qs] for the projection.
            # Returned as closures the NEXT phase emits into its DVE slack.
            def norm_part(j, qq, with_recip, with_transpose, rc_box={}):
                def emit():
                    if os.environ.get("KDBG_NO_NORM"):
                        return
                    if with_recip:
                        rc = rc_pool.tile([128, 2, 2], F32, tag="rc", name="rc")
                        nc.vector.reciprocal(
                            rc[:], cx[j][:, :, HD:2 * (HD + 1):HD + 1]
                        )
                        rc_box[j] = rc
                    rc = rc_box[j]
                    qsb = Q * 4 + j * 2 + qq
                    nc.vector.tensor_mul(
                        ctx_sb[m][:, qsb, :].rearrange("p (h e) -> p h e", h=2),
                        cx[j][:, qq, :].rearrange(
                            "p (h e) -> p h e", e=HD + 1
                        )[:, :, 0:HD],
                        rc[:, qq, :].broadcast_to([128, 2, HD]),
                    )
                    if with_transpose and pe_t:
                        # tail: PE-mode transpose into a freed sc bank + DVE
                        # copy (~1.0us chain) instead of the DMA xbar
                        # transpose's ~2.6us dge+sem latency
                        tp = ps_sc.tile([128, 2, 128], F16, tag="sc", name="tp")
                        for q2 in range(2):
                            nc.tensor.transpose(
                                tp[:, q2, :],
                                ctx_sb[m][:, Q * 4 + 2 * j + q2, :],
                                id_sb[:],
                            )
                        copy(ctxT[m][:, Q * 4 + 2 * j:Q * 4 + 2 * j + 2, :],
                             tp[:], nc.vector if j else nc.scalar)
                        return
                    if with_transpose:
                        if os.environ.get("KDBG_NO_TRANSPOSE"):
                            nc.sync.dma_start(
                                ctxT[m][:, Q * 4 + 2 * j:Q * 4 + 2 * j + 2, :],
                                ctx_sb[m][:, Q * 4 + 2 * j:Q * 4 + 2 * j + 2, :],
                            )
                        else:
                            nc.sync.dma_start_transpose(
                                ctxT[m][:, Q * 4 + 2 * j:Q * 4 + 2 * j + 2, :],
                                ctx_sb[m][:, Q * 4 + 2 * j:Q * 4 + 2 * j + 2, :],
                            )
                return emit

            box = {}
            return [
                norm_part(0, 0, True, False, box),
                norm_part(0, 1, False, True, box),
                norm_part(1, 0, True, False, box),
                norm_part(1, 1, False, True, box),
            ]

        def proj_half(Q, j, tail=False):
            for qsb in range(Q * 4 + 2 * j, Q * 4 + 2 * j + 2):
                stage = st_pool.tile([128, D], F16, tag="st", name="st")
                for nb in range(2):
                    # the final quarter's projections use the freed cx banks
                    # so the tail is double-buffered despite gp bufs=1
                    pso = (
                        ps_cx.tile([128, 512], F32, tag=f"cx{nb}", name="o")
                        if tail
                        else ps_gp.tile([128, 512], F32, tag="gp", name="o")
                    )
                    for m in range(2):
                        nc.tensor.matmul(
                            pso[:],
                            ctxT[m][:, qsb, :],
                            wo_sb[:, m, nb * 512:(nb + 1) * 512],
                            start=(m == 0),
                            stop=(m == 1),
                        )
                    copy(stage[:, nb * 512:(nb + 1) * 512], pso[:],
                         (nc.scalar if nb == 0 else nc.vector)
                         if tail else nc.vector)
                    if tail:
                        nc.sync.dma_start(
                            out[qsb * 128:(qsb + 1) * 128,
                                nb * 512:(nb + 1) * 512],
                            stage[:, nb * 512:(nb + 1) * 512],
                        )
                if not tail:
                    nc.sync.dma_start(out[qsb * 128:(qsb + 1) * 128, :], stage[:])

        # ---- emission order = scheduler priority: the minimum needed for
        # attention (m0, Q0) first, then the deferred q projections and
        # output projections as PE gap filler while exp chains bound the
        # attention phases ----
        # chunk-major so no x-gated group ever sits ahead of ready work;
        # the pre-attention copies run on ACT (its exp chain is idle there)
        qk_group(0, kT[0], 0, 256, eng=nc.scalar, pad_k=True, pre=True)
        qk_group(0, kT[0], 0, 256, 256, eng=nc.scalar, pad_k=True, pre=True)
        qk_group(256, qT[0], 0, 256, eng=nc.scalar, pre=True)
        qk_group(256, qT[0], 0, 256, 256, eng=nc.scalar, pre=True)
        qk_group(128, kT[1], 0, eng=nc.scalar, pad_k=True, pre=True)
        qk_group(384, qT[1], 0, eng=nc.scalar, pre=True)
        for sb in range(4):
            v_group(sb)
        for c in range(1, NQ):
            qk_group(0, kT[0], c, eng=nc.scalar, pad_k=True, pre=True)
            for sb in range(c * 4, c * 4 + 4):
                v_group(sb)
        NPH = int(os.environ.get("KDBG_NPHASES", "8"))
        phase_list = [(0, 0), (1, 0), (0, 1), (1, 1), (0, 2), (1, 2),
                      (0, 3), (1, 3)][:NPH]
        pi = [0]

        def next_phase(norms):
            if pi[0] >= len(phase_list):
                return norms
            m, Q = phase_list[pi[0]]
            pe_t = pi[0] == len(phase_list) - 1
            pi[0] += 1
            return attention_phase(m, Q, norms, pe_t)

        norms = next_phase(())                     # (0,0)
        for c in range(1, NQ):
            qk_group(128, kT[1], c, pad_k=True)
        qk_group(256, qT[0], 1)
        norms = next_phase(norms)                  # (1,0)
        qk_group(384, qT[1], 1)
        norms = next_phase(norms)                  # (0,1)
        if NPH >= 3:
            proj_half(0, 0)
        qk_group(256, qT[0], 2)
        if NPH >= 3:
            proj_half(0, 1)
        for Q in range(1, NQ):
            norms = next_phase(norms)              # (1,Q)
            if Q < NQ - 1:
                qk_group(384, qT[1], Q + 1)
                norms = next_phase(norms)          # (0,Q+1)
                if NPH >= 2 * Q + 3:
                    proj_half(Q, 0)
                if Q < NQ - 2:
                    qk_group(256, qT[0], Q + 2)
                if NPH >= 2 * Q + 3:
                    proj_half(Q, 1)
        for fn in norms:                     # final phase's norms
            fn()
        if NPH >= 8:
            proj_half(NQ - 1, 0, tail=True)
            proj_half(NQ - 1, 1, tail=True)
        ctx.close()
    return nc


_NC_CACHE = None


def _get_nc():
    global _NC_CACHE
    if _NC_CACHE is None:
        _NC_CACHE = build_nc()
    return _NC_CACHE


_EXEC_CACHE = None


def _get_executor():
    """Build + jit the SPMD executable once; reuse across kernel() calls."""
    global _EXEC_CACHE
    if _EXEC_CACHE is not None:
        return _EXEC_CACHE
    import jax
    from jax.sharding import Mesh, PartitionSpec
    from jax.experimental.shard_map import shard_map
    from concourse import bass2jax as b2j

    nc = _get_nc()
    b2j.install_neuronx_cc_hook()
    assert nc.dbg_addr is None
    partition_name = (
        nc.partition_id_tensor.name if nc.partition_id_tensor is not None else None
    )

    in_names, out_names, out_avals = [], [], []
    for alloc in nc.m.functions[0].allocations:
        if not isinstance(alloc, mybir.MemoryLocationSet):
            continue
        name = alloc.memorylocations[0].name
        if alloc.kind == "ExternalInput":
            if name != partition_name:
                in_names.append(name)
        elif alloc.kind == "ExternalOutput":
            out_names.append(name)
            out_avals.append(
                jax.core.ShapedArray(
                    tuple(alloc.tensor_shape), mybir.dt.np(alloc.dtype)
                )
            )
    n_params = len(in_names)
    n_outs = len(out_avals)
    all_names = in_names + out_names
    if partition_name is not None:
        all_names = all_names + [partition_name]

    def _body(*args):
        operands = list(args)
        if partition_name is not None:
            operands.append(b2j.partition_id_tensor())
        outs = b2j._bass_exec_p.bind(
            *operands,
            out_avals=tuple(out_avals),
            in_names=tuple(all_names),
            out_names=tuple(out_names),
            lowering_input_output_aliases=(),
            sim_require_finite=True,
            sim_require_nnan=True,
            nc=nc,
        )
        return tuple(outs)

    devices = jax.devices()[:NCORES]
    mesh = Mesh(np.asarray(devices), ("core",))
    donate = tuple(range(n_params, n_params + n_outs))
    sharded = jax.jit(
        shard_map(
            _body,
            mesh=mesh,
            in_specs=(PartitionSpec("core"),) * (n_params + n_outs),
            out_specs=(PartitionSpec("core"),) * n_outs,
            check_rep=False,
        ),
        donate_argnums=donate,
        keep_unused=True,
    )
    import jax.numpy as jnp

    zero_shardings = [
        jax.sharding.NamedSharding(mesh, PartitionSpec("core"))
    ] * n_outs

    @jax.jit
    def _make_zeros():
        return tuple(
            jax.lax.with_sharding_constraint(
                jnp.zeros((NCORES * a.shape[0], *a.shape[1:]), a.dtype), sh
            )
            for a, sh in zip(out_avals, zero_shardings)
        )

    _EXEC_CACHE = {
        "sharded": sharded,
        "make_zeros": _make_zeros,
        "in_names": in_names,
        "out_names": out_names,
        "out_avals": out_avals,
    }
    return _EXEC_CACHE


def _run_spmd(in_maps):
    ex = _get_executor()
    concat_in = [
        np.concatenate([np.asarray(m[name]) for m in in_maps], axis=0)
        for name in ex["in_names"]
    ]
    concat_zeros = ex["make_zeros"]()
    out_arrs = ex["sharded"](*concat_in, *concat_zeros)
    results = []
    for c in range(NCORES):
        results.append({
            name: np.asarray(out_arrs[i]).reshape(
                NCORES, *ex["out_avals"][i].shape
            )[c]
            for i, name in enumerate(ex["out_names"])
        })
    return results


def _shard_inputs(x, Wq, Wk, Wv, Wo):
    scale = np.float32(1.0 / np.sqrt(HD))
    in_maps = []
    xT_b = [np.ascontiguousarray(x[b].T).astype(np.float16) for b in range(B)]
    for c in range(NCORES):
        b, g = divmod(c, GROUPS)
        sl = slice(g * E, (g + 1) * E)
        wq = (Wq[sl, :] * scale).T.astype(np.float16)   # [D, 256]
        wk = Wk[sl, :].T.astype(np.float16)
        wv = Wv[sl, :].T.astype(np.float16)
        # k pair0 | k pair1 | q pair0 | q pair1 | v
        wqkv = np.concatenate([wk, wq, wv], axis=1)
        in_maps.append({
            "xT": xT_b[b],
            "wqkv": np.ascontiguousarray(wqkv),
            "woT": np.ascontiguousarray(Wo[:, sl].T).astype(np.float16),
            "ident": np.eye(128, dtype=np.float16),
        })
    return in_maps


_FAST_PATH_OK = True


def kernel(x, Wq, Wk, Wv, Wo, bo):
    global _FAST_PATH_OK
    x = np.asarray(x, dtype=np.float32)
    in_maps = _shard_inputs(
        x,
        np.asarray(Wq, dtype=np.float32),
        np.asarray(Wk, dtype=np.float32),
        np.asarray(Wv, dtype=np.float32),
        np.asarray(Wo, dtype=np.float32),
    )
    results = None
    if _FAST_PATH_OK:
        try:
            results = _run_spmd(in_maps)
        except Exception:
            _FAST_PATH_OK = False
    if results is None:
        results = run_bass_kernel_spmd(
            _get_nc(), in_maps, list(range(NCORES))
        ).results
    bo = np.asarray(bo, dtype=np.float32)
    out = np.empty((B, S, D), dtype=np.float32)
    for b in range(B):
        acc = np.zeros((S, D), dtype=np.float64)
        for g in range(GROUPS):
            acc += results[b * GROUPS + g]["out_partial"]
        out[b] = (acc + bo.astype(np.float64)).astype(np.float32)
    return out
